# revision 5
# baseline (speedup 1.0000x reference)
"""Trainium2 Bass kernel for capsule dynamic routing (nn_Capsule).

Math (per sample):
  hat[i,(n,d)] = sum_d' x[i,d'] W[d',(n,d)]        (i=1024, d'=128, n=32, d=16)
  3 routing iters: c = softmax(b, axis=n); o = squash(sum_i c[n,i] hat[i,n,:])
                   b = sum_d o[n,d] hat[i,n,d]
Key refactor -- never materialize hat (256 MiB):
  step b: s[n,d]  = sum_d' G[n,d'] W[d',(n,d)],  G = c @ x        (32x128)
  step c: b[n,i]  = sum_d' x[i,d'] H[n,d'],      H = oM @ W^T     (32x128)
          where oM[n',(n,d)] = o[n,d] * (n==n')  (block-diag masked o)
Sharding: data-parallel over batch, 16 samples/core x 8 cores.
Samples are processed in groups of 4, stacked on the partition dim (4*32=128)
so the small routing matrices use the full PE/DVE width.
"""

import os
import sys

sys.path.insert(0, "/opt/trn_rl_repo")

import numpy as np

import concourse.bass as bass
import concourse.bacc as bacc
import concourse.mybir as mybir
from concourse import tile
from concourse.bass_utils import run_bass_kernel_spmd

FP32 = mybir.dt.float32
AF = mybir.ActivationFunctionType
AX = mybir.AxisListType

EPS = 1e-7
N_CORES = 8
B_TOTAL, IN, D = 128, 1024, 128
NCAP, DC = 32, 16
K = NCAP * DC          # 512
B_LOC = B_TOTAL // N_CORES   # 16 samples per core
GSZ = 4                # samples batched per group (4*32 = 128 partitions)
NG = B_LOC // GSZ      # 4 groups
NCH = IN // 128        # 8 chunks of the In dimension


def build():
    nc = bacc.Bacc("TRN2", target_bir_lowering=False)
    xT = nc.declare_dram_parameter("xT", [B_LOC, D, IN], FP32, isOutput=False)
    xn = nc.declare_dram_parameter("xn", [B_LOC, IN, D], FP32, isOutput=False)
    w = nc.declare_dram_parameter("w", [D, K], FP32, isOutput=False)
    wt = nc.declare_dram_parameter("wt", [K, D], FP32, isOutput=False)
    mask4 = nc.declare_dram_parameter("mask4", [GSZ * NCAP, K], FP32, isOutput=False)
    ident = nc.declare_dram_parameter("ident", [128, 128], FP32, isOutput=False)
    out = nc.declare_dram_parameter("out", [B_LOC, NCAP, DC], FP32, isOutput=True)

    with tile.TileContext(nc) as tc:
        with (
            tc.tile_pool(name="const", bufs=1) as cpool,
            tc.tile_pool(name="xTp", bufs=2 * GSZ) as xTp,
            tc.tile_pool(name="xnp", bufs=2 * GSZ) as xnp,
            tc.tile_pool(name="sb128", bufs=4) as sb128,
            tc.tile_pool(name="sbbig", bufs=3) as sbbig,
            tc.tile_pool(name="small", bufs=8) as smallp,
            tc.tile_pool(name="softp", bufs=2 * GSZ) as softp,
            tc.tile_pool(name="mm128", bufs=3, space="PSUM") as mm128,
            tc.tile_pool(name="fps", bufs=1, space="PSUM") as fps,
            tc.tile_pool(name="omtps", bufs=2, space="PSUM") as omtps,
            tc.tile_pool(name="btps", bufs=2, space="PSUM") as btps,
        ):
            w_sb = cpool.tile([D, K], FP32, tag="w")
            nc.sync.dma_start(w_sb[:], w[:])
            wt_sb = cpool.tile([128, 4, D], FP32, tag="wt")
            nc.sync.dma_start(wt_sb[:], wt.rearrange("(j p) d -> p j d", p=128))
            m4_sb = cpool.tile([128, K], FP32, tag="mask4")
            nc.sync.dma_start(m4_sb[:], mask4[:])
            id_sb = cpool.tile([128, 128], FP32, tag="ident")
            nc.sync.dma_start(id_sb[:], ident[:])
            c0_sb = cpool.tile([128, NCH * NCAP], FP32, tag="c0")
            nc.vector.memset(c0_sb[:], 1.0 / NCAP)
            eps_sb = cpool.tile([128, 1], FP32, tag="eps")
            nc.vector.memset(eps_sb[:], EPS)
            zero_sb = cpool.tile([128, 1], FP32, tag="zero")
            nc.vector.memset(zero_sb[:], 0.0)
            c0_v = c0_sb[:].rearrange("p (c n) -> p c n", c=NCH)

            for g in range(NG):
                xT_t, xn_t = [], []
                for b in range(GSZ):
                    bb = g * GSZ + b
                    t = xTp.tile([128, IN], FP32, tag="xT")
                    nc.sync.dma_start(t[:], xT[bb])
                    xT_t.append(t)
                    t2 = xnp.tile([128, NCH, D], FP32, tag="xn")
                    nc.sync.dma_start(t2[:], xn[bb].rearrange("(c p) d -> p c d", p=128))
                    xn_t.append(t2)

                cT = [None] * GSZ
                for it in range(3):
                    # ---- step b: G = c @ x, 4 samples stacked on partitions ----
                    G4 = mm128.tile([128, 128], FP32, tag="mm128")
                    for b in range(GSZ):
                        csrc = c0_v if it == 0 else cT[b][:]
                        for c in range(NCH):
                            nc.tensor.matmul(
                                G4[b * NCAP:(b + 1) * NCAP, :],
                                csrc[:, c, :],
                                xn_t[b][:, c, :],
                                start=(c == 0),
                                stop=(c == NCH - 1),
                                tile_position=(0, b * NCAP),
                            )
                    Gs4 = sb128.tile([128, 128], FP32, tag="sb128")
                    nc.scalar.copy(Gs4[:], G4[:])
                    GT4 = mm128.tile([128, 128], FP32, tag="mm128")
                    nc.tensor.transpose(GT4[:], Gs4[:], id_sb[:])
                    GTs4 = sb128.tile([128, 128], FP32, tag="sb128")
                    nc.scalar.copy(GTs4[:], GT4[:])
                    # F4[(b,n'),(n,d)] = sum_d' G[(b,n'),d'] W[d',(n,d)]
                    F4 = fps.tile([128, K], FP32, tag="f4")
                    nc.tensor.matmul(F4[:], GTs4[:], w_sb[:], start=True, stop=True)
                    # keep only diagonal capsule blocks, reduce over n
                    ts4 = sbbig.tile([128, K], FP32, tag="ts4")
                    nc.vector.tensor_mul(ts4[:], F4[:], m4_sb[:])
                    s4 = smallp.tile([128, DC], FP32, tag="s4")
                    nc.vector.reduce_sum(
                        s4[:], ts4[:].rearrange("p (n d) -> p d n", n=NCAP), axis=AX.X
                    )
                    # squash: scale = sqrt(ss+EPS) / (0.5 + ss+EPS)
                    sq4 = smallp.tile([128, DC], FP32, tag="sq4")
                    nc.scalar.activation(sq4[:], s4[:], AF.Square, bias=zero_sb[:])
                    ss4 = smallp.tile([128, 1], FP32, tag="ss4")
                    nc.vector.reduce_sum(ss4[:], sq4[:], axis=AX.X)
                    num4 = smallp.tile([128, 1], FP32, tag="num4")
                    nc.scalar.activation(num4[:], ss4[:], AF.Sqrt, bias=eps_sb[:])
                    den4 = smallp.tile([128, 1], FP32, tag="den4")
                    nc.vector.tensor_scalar_add(den4[:], ss4[:], 0.5 + EPS)
                    rden4 = smallp.tile([128, 1], FP32, tag="rden4")
                    nc.vector.reciprocal(rden4[:], den4[:])
                    scale4 = smallp.tile([128, 1], FP32, tag="scale4")
                    nc.vector.tensor_mul(scale4[:], num4[:], rden4[:])

                    if it == 2:
                        o4 = smallp.tile([128, DC], FP32, tag="o4")
                        nc.vector.tensor_scalar_mul(o4[:], s4[:], scale4[:])
                        nc.sync.dma_start(
                            out[g * GSZ:(g + 1) * GSZ].rearrange("b n d -> (b n) d"),
                            o4[:],
                        )
                        continue

                    # ---- step c: H = oM @ W^T then b' = xT.T @ H^T ----
                    oM4 = sbbig.tile([128, K], FP32, tag="oM4")
                    nc.vector.tensor_scalar_mul(oM4[:], ts4[:], scale4[:])
                    H4 = mm128.tile([128, 128], FP32, tag="mm128")
                    for j in range(4):
                        oMT = omtps.tile([128, 128], FP32, tag="omt")
                        nc.tensor.transpose(
                            oMT[:], oM4[:, j * 128:(j + 1) * 128], id_sb[:]
                        )
                        oMTs = sb128.tile([128, 128], FP32, tag="sb128")
                        nc.scalar.copy(oMTs[:], oMT[:])
                        nc.tensor.matmul(
                            H4[:], oMTs[:], wt_sb[:, j, :], start=(j == 0), stop=(j == 3)
                        )
                    Hs4 = sb128.tile([128, 128], FP32, tag="sb128")
                    nc.scalar.copy(Hs4[:], H4[:])
                    HT4 = mm128.tile([128, 128], FP32, tag="mm128")
                    nc.tensor.transpose(HT4[:], Hs4[:], id_sb[:])
                    HTs4 = sb128.tile([128, 128], FP32, tag="sb128")
                    nc.scalar.copy(HTs4[:], HT4[:])

                    for b in range(GSZ):
                        bt = btps.tile([128, NCH * NCAP], FP32, tag="bt")
                        for c in range(NCH):
                            nc.tensor.matmul(
                                bt[:, c * NCAP:(c + 1) * NCAP],
                                xT_t[b][:, c * 128:(c + 1) * 128],
                                HTs4[:, b * NCAP:(b + 1) * NCAP],
                                start=True,
                                stop=True,
                            )
                        # softmax over n (free dim), no max-subtraction needed:
                        # |b| <= |o||hat_i| which is O(10) here, exp is safe in fp32
                        e = softp.tile([128, NCH, NCAP], FP32, tag="e")
                        nc.scalar.activation(
                            e[:].rearrange("p c n -> p (c n)"), bt[:], AF.Exp,
                            bias=zero_sb[:],
                        )
                        z = smallp.tile([128, NCH], FP32, tag="z")
                        nc.vector.reduce_sum(z[:], e[:], axis=AX.X)
                        rz = smallp.tile([128, NCH], FP32, tag="rz")
                        nc.vector.reciprocal(rz[:], z[:])
                        ct = softp.tile([128, NCH, NCAP], FP32, tag="ct")
                        for c in range(NCH):
                            nc.vector.tensor_scalar_mul(
                                ct[:, c, :], e[:, c, :], rz[:, c:c + 1]
                            )
                        cT[b] = ct
    nc.compile()
    return nc


LAST_RESULT = None
_MASK = None


def _consts():
    global _MASK
    if _MASK is None:
        m = np.zeros((NCAP, K), np.float32)
        for n in range(NCAP):
            m[n, n * DC:(n + 1) * DC] = 1.0
        _MASK = np.tile(m, (GSZ, 1))
    return _MASK


def kernel(inputs, kernel):
    x = np.ascontiguousarray(np.asarray(inputs, dtype=np.float32))
    W = np.ascontiguousarray(np.asarray(kernel, dtype=np.float32)[0])
    xTh = np.ascontiguousarray(x.transpose(0, 2, 1))
    WT = np.ascontiguousarray(W.T)
    mask4 = _consts()
    identity = np.eye(128, dtype=np.float32)

    nc = build()
    in_maps = [
        {
            "xT": xTh[i * B_LOC:(i + 1) * B_LOC],
            "xn": x[i * B_LOC:(i + 1) * B_LOC],
            "w": W,
            "wt": WT,
            "mask4": mask4,
            "ident": identity,
        }
        for i in range(N_CORES)
    ]
    res = run_bass_kernel_spmd(
        nc, in_maps, core_ids=list(range(N_CORES)),
        trace=bool(os.environ.get("KERNEL_TRACE")),
    )
    global LAST_RESULT
    LAST_RESULT = res
    return np.concatenate([res.results[i]["out"] for i in range(N_CORES)], axis=0)


if __name__ == "__main__":
    rng = np.random.default_rng(0)
    xi = rng.standard_normal((B_TOTAL, IN, D), dtype=np.float32)
    ki = (rng.standard_normal((1, D, K), dtype=np.float32) * 0.05).astype(np.float32)
    o = kernel(xi, ki)
    print(o.shape, o.dtype)


# revision 6
# speedup vs baseline: 1.7290x; 1.7290x over previous
"""Trainium2 Bass kernel for capsule dynamic routing (nn_Capsule).

Math (per sample):
  hat[i,(n,d)] = sum_d' x[i,d'] W[d',(n,d)]        (i=1024, d'=128, n=32, d=16)
  3 routing iters: c = softmax(b, axis=n); o = squash(sum_i c[n,i] hat[i,n,:])
                   b = sum_d o[n,d] hat[i,n,d]
Key refactor -- never materialize hat (256 MiB):
  step b: s[n,d]  = sum_d' G[n,d'] W[d',(n,d)],  G = c @ x        (32x128)
  step c: b[n,i]  = sum_d' x[i,d'] H[n,d'],      H = oM @ W^T     (32x128)
          where oM[n',(n,d)] = o[n,d] * (n==n')  (block-diag masked o)
Sharding: data-parallel over batch, 16 samples/core x 8 cores.
Samples are processed in groups of 4, stacked on the partition dim (4*32=128)
so the small routing matrices use the full PE/DVE width.
"""

import os
import sys

sys.path.insert(0, "/opt/trn_rl_repo")

import numpy as np

import concourse.bass as bass
import concourse.bacc as bacc
import concourse.mybir as mybir
from concourse import tile
from concourse.bass_utils import run_bass_kernel_spmd

FP32 = mybir.dt.float32
BF16 = mybir.dt.bfloat16
AF = mybir.ActivationFunctionType
AX = mybir.AxisListType

EPS = 1e-7
N_CORES = 8
B_TOTAL, IN, D = 128, 1024, 128
NCAP, DC = 32, 16
K = NCAP * DC          # 512
B_LOC = B_TOTAL // N_CORES   # 16 samples per core
GSZ = 4                # samples batched per group (4*32 = 128 partitions)
NG = B_LOC // GSZ      # 4 groups
NCH = IN // 128        # 8 chunks of the In dimension


def build():
    nc = bacc.Bacc("TRN2", target_bir_lowering=False)
    xT = nc.declare_dram_parameter("xT", [B_LOC, D, IN], BF16, isOutput=False)
    xn = nc.declare_dram_parameter("xn", [B_LOC, 128, NCH, D], BF16, isOutput=False)
    w = nc.declare_dram_parameter("w", [D, K], FP32, isOutput=False)
    wt = nc.declare_dram_parameter("wt", [K, D], FP32, isOutput=False)
    mask4 = nc.declare_dram_parameter("mask4", [GSZ * NCAP, K], FP32, isOutput=False)
    ident = nc.declare_dram_parameter("ident", [128, 128], FP32, isOutput=False)
    out = nc.declare_dram_parameter("out", [B_LOC, NCAP, DC], FP32, isOutput=True)

    with tile.TileContext(nc) as tc:
        with (
            tc.tile_pool(name="const", bufs=1) as cpool,
            tc.tile_pool(name="xTp", bufs=2 * GSZ) as xTp,
            tc.tile_pool(name="xnp", bufs=2 * GSZ) as xnp,
            tc.tile_pool(name="sb128", bufs=4) as sb128,
            tc.tile_pool(name="sbbig", bufs=3) as sbbig,
            tc.tile_pool(name="small", bufs=8) as smallp,
            tc.tile_pool(name="softp", bufs=2 * GSZ) as softp,
            tc.tile_pool(name="mm128", bufs=2, space="PSUM") as mm128,
            tc.tile_pool(name="fps", bufs=1, space="PSUM") as fps,
            tc.tile_pool(name="omtps", bufs=1, space="PSUM") as omtps,
            tc.tile_pool(name="btps", bufs=2, space="PSUM") as btps,
        ):
            w_sb = cpool.tile([D, K], FP32, tag="w")
            nc.sync.dma_start(w_sb[:], w[:])
            wt_sb = cpool.tile([128, 4, D], FP32, tag="wt")
            nc.sync.dma_start(wt_sb[:], wt.rearrange("(j p) d -> p j d", p=128))
            m4_sb = cpool.tile([128, K], FP32, tag="mask4")
            nc.sync.dma_start(m4_sb[:], mask4[:])
            id_sb = cpool.tile([128, 128], FP32, tag="ident")
            nc.sync.dma_start(id_sb[:], ident[:])
            c0_sb = cpool.tile([128, NCH * NCAP], BF16, tag="c0")
            nc.vector.memset(c0_sb[:], 1.0 / NCAP)
            eps_sb = cpool.tile([128, 1], FP32, tag="eps")
            nc.vector.memset(eps_sb[:], EPS)
            zero_sb = cpool.tile([128, 1], FP32, tag="zero")
            nc.vector.memset(zero_sb[:], 0.0)
            c0_v = c0_sb[:].rearrange("p (c n) -> p c n", c=NCH)

            for g in range(NG):
                xT_t, xn_t = [], []
                for b in range(GSZ):
                    bb = g * GSZ + b
                    t = xTp.tile([128, IN], BF16, tag="xT")
                    nc.sync.dma_start(t[:], xT[bb])
                    xT_t.append(t)
                    t2 = xnp.tile([128, NCH, D], BF16, tag="xn")
                    nc.sync.dma_start(t2[:], xn[bb])
                    xn_t.append(t2)

                cT = [None] * GSZ
                for it in range(3):
                    # ---- step b: GT[d',(b,n)] = sum_i x[i,d'] c[n,i] directly ----
                    GT4 = mm128.tile([128, GSZ, NCAP], FP32, tag="mm128")
                    for b in range(GSZ):
                        csrc = c0_v if it == 0 else cT[b][:]
                        for c in range(NCH):
                            nc.tensor.matmul(
                                GT4[:, b, :],
                                xn_t[b][:, c, :],
                                csrc[:, c, :],
                                start=(c == 0),
                                stop=(c == NCH - 1),
                            )
                    GTs4 = sb128.tile([128, 128], FP32, tag="sb128")
                    nc.scalar.copy(GTs4[:], GT4[:].rearrange("p b n -> p (b n)"))
                    # F4[(b,n'),(n,d)] = sum_d' G[(b,n'),d'] W[d',(n,d)]
                    F4 = fps.tile([128, K], FP32, tag="f4")
                    nc.tensor.matmul(F4[:], GTs4[:], w_sb[:], start=True, stop=True)
                    # keep only diagonal capsule blocks, reduce over n
                    ts4 = sbbig.tile([128, K], FP32, tag="ts4")
                    nc.vector.tensor_mul(ts4[:], F4[:], m4_sb[:])
                    s4 = smallp.tile([128, DC], FP32, tag="s4")
                    nc.vector.reduce_sum(
                        s4[:], ts4[:].rearrange("p (n d) -> p d n", n=NCAP), axis=AX.X
                    )
                    # squash: scale = sqrt(ss+EPS) / (0.5 + ss+EPS)
                    sq4 = smallp.tile([128, DC], FP32, tag="sq4")
                    nc.scalar.activation(sq4[:], s4[:], AF.Square, bias=zero_sb[:])
                    ss4 = smallp.tile([128, 1], FP32, tag="ss4")
                    nc.vector.reduce_sum(ss4[:], sq4[:], axis=AX.X)
                    num4 = smallp.tile([128, 1], FP32, tag="num4")
                    nc.scalar.activation(num4[:], ss4[:], AF.Sqrt, bias=eps_sb[:])
                    den4 = smallp.tile([128, 1], FP32, tag="den4")
                    nc.vector.tensor_scalar_add(den4[:], ss4[:], 0.5 + EPS)
                    rden4 = smallp.tile([128, 1], FP32, tag="rden4")
                    nc.vector.reciprocal(rden4[:], den4[:])
                    scale4 = smallp.tile([128, 1], FP32, tag="scale4")
                    nc.vector.tensor_mul(scale4[:], num4[:], rden4[:])

                    if it == 2:
                        o4 = smallp.tile([128, DC], FP32, tag="o4")
                        nc.vector.tensor_scalar_mul(o4[:], s4[:], scale4[:])
                        nc.sync.dma_start(
                            out[g * GSZ:(g + 1) * GSZ].rearrange("b n d -> (b n) d"),
                            o4[:],
                        )
                        continue

                    # ---- step c: H = oM @ W^T then b' = xT.T @ H^T ----
                    oM4 = sbbig.tile([128, K], FP32, tag="oM4")
                    nc.vector.tensor_scalar_mul(oM4[:], ts4[:], scale4[:])
                    H4 = mm128.tile([128, 128], FP32, tag="mm128")
                    for j in range(4):
                        oMT = omtps.tile([128, 128], FP32, tag="omt")
                        nc.tensor.transpose(
                            oMT[:], oM4[:, j * 128:(j + 1) * 128], id_sb[:]
                        )
                        oMTs = sb128.tile([128, 128], FP32, tag="sb128")
                        nc.scalar.copy(oMTs[:], oMT[:])
                        nc.tensor.matmul(
                            H4[:], oMTs[:], wt_sb[:, j, :], start=(j == 0), stop=(j == 3)
                        )
                    Hs4 = sb128.tile([128, 128], FP32, tag="sb128")
                    nc.scalar.copy(Hs4[:], H4[:])
                    HT4 = mm128.tile([128, 128], FP32, tag="mm128")
                    nc.tensor.transpose(HT4[:], Hs4[:], id_sb[:])
                    HTs4 = sb128.tile([128, 128], BF16, tag="sbh")
                    nc.scalar.copy(HTs4[:], HT4[:])

                    for b in range(GSZ):
                        bt = btps.tile([128, NCH * NCAP], FP32, tag="bt")
                        for c in range(NCH):
                            nc.tensor.matmul(
                                bt[:, c * NCAP:(c + 1) * NCAP],
                                xT_t[b][:, c * 128:(c + 1) * 128],
                                HTs4[:, b * NCAP:(b + 1) * NCAP],
                                start=True,
                                stop=True,
                            )
                        # softmax over n (free dim), no max-subtraction needed:
                        # |b| <= |o||hat_i| which is O(10) here, exp is safe in fp32
                        e = softp.tile([128, NCH, NCAP], FP32, tag="e")
                        nc.scalar.activation(
                            e[:].rearrange("p c n -> p (c n)"), bt[:], AF.Exp,
                            bias=zero_sb[:],
                        )
                        z = smallp.tile([128, NCH], FP32, tag="z")
                        nc.vector.reduce_sum(z[:], e[:], axis=AX.X)
                        rz = smallp.tile([128, NCH], FP32, tag="rz")
                        nc.vector.reciprocal(rz[:], z[:])
                        ct = softp.tile([128, NCH, NCAP], BF16, tag="ct")
                        for c in range(NCH):
                            nc.vector.tensor_scalar_mul(
                                ct[:, c, :], e[:, c, :], rz[:, c:c + 1]
                            )
                        cT[b] = ct
    nc.compile()
    return nc


LAST_RESULT = None
_MASK = None


def _consts():
    global _MASK
    if _MASK is None:
        m = np.zeros((NCAP, K), np.float32)
        for n in range(NCAP):
            m[n, n * DC:(n + 1) * DC] = 1.0
        _MASK = np.tile(m, (GSZ, 1))
    return _MASK


def kernel(inputs, kernel):
    import ml_dtypes
    bf16 = ml_dtypes.bfloat16
    x = np.ascontiguousarray(np.asarray(inputs, dtype=np.float32))
    W = np.ascontiguousarray(np.asarray(kernel, dtype=np.float32)[0])
    xTh = np.ascontiguousarray(x.transpose(0, 2, 1).astype(bf16))
    # chunk-major natural layout: xnL[b, p, c, d] = x[b, c*128+p, d]
    # -> per-sample DMA rows are 8*128 contiguous elements
    xnL = np.ascontiguousarray(
        x.reshape(B_TOTAL, NCH, 128, D).transpose(0, 2, 1, 3).astype(bf16)
    )
    WT = np.ascontiguousarray(W.T)
    mask4 = _consts()
    identity = np.eye(128, dtype=np.float32)

    nc = build()
    in_maps = [
        {
            "xT": xTh[i * B_LOC:(i + 1) * B_LOC],
            "xn": xnL[i * B_LOC:(i + 1) * B_LOC],
            "w": W,
            "wt": WT,
            "mask4": mask4,
            "ident": identity,
        }
        for i in range(N_CORES)
    ]
    res = run_bass_kernel_spmd(
        nc, in_maps, core_ids=list(range(N_CORES)),
        trace=bool(os.environ.get("KERNEL_TRACE")),
    )
    global LAST_RESULT
    LAST_RESULT = res
    return np.concatenate([res.results[i]["out"] for i in range(N_CORES)], axis=0)


if __name__ == "__main__":
    rng = np.random.default_rng(0)
    xi = rng.standard_normal((B_TOTAL, IN, D), dtype=np.float32)
    ki = (rng.standard_normal((1, D, K), dtype=np.float32) * 0.05).astype(np.float32)
    o = kernel(xi, ki)
    print(o.shape, o.dtype)
